# revision 7
# baseline (speedup 1.0000x reference)
"""CapsMaxPool Trainium2 kernel.

x: [B=64, H=64, W=64, C=32, A=8] fp32 capsules. For each 2x2 spatial window
and capsule c, pick the candidate position whose capsule vector has the
largest L2 norm (first-max-wins over the window in row-major (ph, pw) order)
and emit that 8-atom vector. Output: [B, 32, 32, 32, 8].

Strategy (per core; batch sharded 8 ways -> 8 examples/core):
  - Layout: spatial rows on SBUF partitions, (capsule, atom) on the free dim.
    One partition row = one (b, ho) output row.
  - The four window candidates are DMA'd as slices of one SBUF tile; the
    paired load rides both pw candidates in one transfer (2 KiB contiguous
    DRAM chunks).
  - ScalarE squares everything (fp32; fp16/bf16 squares were measured to
    flip 346/2691 argmax decisions vs the fp32 reference, while the fp32
    pipeline is bit-exact and stable to +-6 ulp of any summation order).
  - Atom sums via a pairwise add tree on VectorE (tensor-tensor adds read
    2 elems/cycle via two SBUF ports vs tensor_reduce's 1).
  - VectorE builds strictly-greater masks against the running max
    (first-max-wins, identical to jnp.argmax), then copy_predicated
    overwrites the candidate-0 slice in place, broadcasting each (wo, c)
    mask over the 8 atoms via a step-0 AP dim.
"""

import numpy as np

import concourse.bass as bass
import concourse.tile as tile
from concourse import mybir
from concourse.bass_utils import run_bass_kernel_spmd

B, H, W, C, A = 64, 64, 64, 32, 8
PH = PW = 2
NCORES = 8
BL = B // NCORES          # batches per core
Ho, Wo = H // PH, W // PW
CA = C * A                # 256
R = BL * Ho               # 256 partition rows per core ((b, ho) flattened)
WCH = 4                   # wo per tile group
NWCH = Wo // WCH
NRT = R // 128            # row tiles

F32 = mybir.dt.float32


def _split_multi_waits(nc):
    """Walrus on this toolchain encodes at most ONE sync wait per
    instruction; Tile attaches several. Hoist all-but-one wait into
    standalone InstEventSemaphore ops just before the instruction (same
    engine stream position => identical semantics)."""
    for fn in nc.m.functions:
        for bb in fn.blocks:
            new = []
            for ins in bb.instructions:
                si = ins.sync_info
                if si is not None and si.on_wait and len(si.on_wait) > 1:
                    waits = list(si.on_wait)
                    for j, w in enumerate(waits[:-1]):
                        new.append(mybir.InstEventSemaphore(
                            name=f"{ins.name}-hw{j}",
                            engine=ins.engine,
                            ins=[], outs=[],
                            sync_info=mybir.SyncInfo(on_wait=[w], on_update=[]),
                        ))
                    ins.sync_info = mybir.SyncInfo(
                        on_wait=[waits[-1]], on_update=list(si.on_update)
                    )
                new.append(ins)
            bb.instructions = new


def _bcast_atoms(ap):
    """View an AP with an extra step-0 trailing dim of size A."""
    return bass.AP(tensor=ap.tensor, offset=ap.offset, ap=list(ap.ap) + [[0, A]])


def _group(nc, big, small, xv, ov, r0, w0, cfg, dst_out=None):
    NP = PH * PW
    if cfg.get("contig"):
        # One DMA per group: each partition row loads its full window span
        # (both ph rows x 2*WCH w positions) as two contiguous 8 KiB DRAM
        # chunks. Candidates are strided views of the loaded tile.
        xvc = cfg["xvc"]
        xq = big.tile(
            [128, PH, 2 * WCH, CA], F32, name="xq",
            bufs=cfg.get("load_bufs"),
        )
        if cfg.get("split4"):
            # 4 DMAs: one per (ph, w-half) — 4 KiB contiguous chunks with
            # candidate-level dependency granularity. Optionally spread
            # across both HWDGE rings (SP + Activation).
            for ph in range(PH):
                for wh in range(2):
                    eng = (
                        nc.scalar
                        if cfg.get("in_split_rings") and wh == 1
                        else nc.sync
                    )
                    eng.dma_start(
                        out=xq[:, ph, wh * WCH : (wh + 1) * WCH, :],
                        in_=xvc[
                            r0 : r0 + 128, ph,
                            2 * w0 + wh * WCH : 2 * w0 + (wh + 1) * WCH, :,
                        ],
                    )
        elif cfg.get("split_ph"):
            for ph in range(PH):
                nc.sync.dma_start(
                    out=xq[:, ph],
                    in_=xvc[r0 : r0 + 128, ph, 2 * w0 : 2 * (w0 + WCH), :],
                )
        else:
            nc.sync.dma_start(
                out=xq,
                in_=xvc[r0 : r0 + 128, :, 2 * w0 : 2 * (w0 + WCH), :],
            )
        xqv = xq[:].rearrange("p ph (wo pw) ca -> p ph pw wo ca", pw=PW)
        cand = lambda p: xqv[:, p // PW, p % PW]
    elif cfg.get("pair_load"):
        # xc laid out [part, w, cand, ca] so one DMA per ph row carries both
        # pw candidates: the DRAM side is then fully contiguous 8 KiB per
        # partition row, and the SBUF side stays a 3-dim AP.
        xcw = big.tile([128, WCH, NP, CA], F32, name="xcw")
        for ph in range(PH):
            nc.sync.dma_start(
                out=xcw[:, :, 2 * ph : 2 * ph + 2, :],
                in_=xv[r0 : r0 + 128, ph, :, w0 : w0 + WCH, :].rearrange(
                    "p pw w ca -> p w pw ca"
                ),
            )
        cand = lambda p: xcw[:, :, p, :]
    else:
        xc = big.tile(
            [128, NP, WCH, CA], F32, name="xc", bufs=cfg.get("load_bufs")
        )
        for p in range(NP):
            ph, pw = divmod(p, PW)
            nc.sync.dma_start(
                out=xc[:, p],
                in_=xv[r0 : r0 + 128, ph, pw, w0 : w0 + WCH, :],
            )
        cand = lambda p: xc[:, p]

    if cfg.get("loads_only"):
        return

    ODT = cfg.get("_odt", F32)

    if cfg.get("dma_only"):
        out_t = big.tile([128, WCH, CA], ODT, name="out_t")
        nc.scalar.copy(out_t, cand(0))
        oe = nc.scalar if cfg.get("out_act_ring") else nc.sync
        oe.dma_start(out=ov[r0 : r0 + 128, w0 : w0 + WCH, :], in_=out_t)
        return

    sq = big.tile(
        [128, NP, WCH, CA], F32, name="sq", bufs=cfg.get("sq_bufs")
    )
    for p in range(NP):
        nc.scalar.activation(
            sq[:, p], cand(p), mybir.ActivationFunctionType.Square
        )

    s = small.tile([128, NP, WCH, C], F32, name="s")
    norm = cfg.get("norm", "tree_dve")
    if norm in ("tree_dve", "tree_gps"):
        l1_eng = nc.gpsimd if norm == "tree_gps" else nc.vector
        sqv = sq[:].rearrange(
            "p q w (c a2 two) -> p q w c a2 two", a2=A // 2, two=2
        )
        t4 = small.tile([128, NP, WCH, C, A // 2], F32, name="t4")
        l1_eng.tensor_add(t4, sqv[:, :, :, :, :, 0], sqv[:, :, :, :, :, 1])
        t4v = t4[:].rearrange("p q w c (b2 two) -> p q w c b2 two", two=2)
        t2 = small.tile([128, NP, WCH, C, A // 4], F32, name="t2")
        nc.vector.tensor_add(t2, t4v[:, :, :, :, :, 0], t4v[:, :, :, :, :, 1])
        t2v = t2[:].rearrange("p q w c (b1 two) -> p q w c b1 two", two=2)
        nc.vector.tensor_add(s, t2v[:, :, :, :, 0, 0], t2v[:, :, :, :, 0, 1])
    else:  # plain reduce on DVE
        nc.vector.tensor_reduce(
            s,
            sq[:].rearrange("p q w (c a) -> p q w c a", a=A),
            axis=mybir.AxisListType.X,
            op=mybir.AluOpType.add,
        )

    # Strict-greater masks vs the running max -> first-max-wins.
    m = [
        small.tile([128, WCH, C], mybir.dt.uint8, name=f"mask{i}", tag=f"mask{i}")
        for i in range(3)
    ]
    r01 = small.tile([128, WCH, C], F32, name="r01")
    r012 = small.tile([128, WCH, C], F32, name="r012")
    nc.vector.tensor_tensor(m[0], s[:, 1], s[:, 0], mybir.AluOpType.is_gt)
    nc.vector.tensor_max(r01, s[:, 0], s[:, 1])
    nc.vector.tensor_tensor(m[1], s[:, 2], r01, mybir.AluOpType.is_gt)
    nc.vector.tensor_max(r012, r01, s[:, 2])
    nc.vector.tensor_tensor(m[2], s[:, 3], r012, mybir.AluOpType.is_gt)

    # Select: overwrite the baseline wherever a later candidate strictly
    # beats the running max (mask broadcast over the 8 atoms via step-0).
    if dst_out is not None:
        dst_slice = dst_out
        nc.scalar.copy(dst_slice, cand(0))
    elif cfg.get("inplace"):
        dst_slice = cand(0)
    else:
        out_t = big.tile([128, WCH, CA], ODT, name="out_t")
        nc.scalar.copy(out_t, cand(0))
        dst_slice = out_t[:]
    dst = dst_slice.rearrange("p w (c a) -> p w c a", a=A)
    for p in range(1, NP):
        nc.vector.copy_predicated(
            dst,
            _bcast_atoms(m[p - 1][:]),
            cand(p).rearrange("p w (c a) -> p w c a", a=A),
        )

    if dst_out is None:
        out_eng = nc.scalar if cfg.get("out_act_ring") else nc.sync
        out_eng.dma_start(
            out=ov[r0 : r0 + 128, w0 : w0 + WCH, :], in_=dst_slice
        )


DEFAULT_CFG = dict(
    norm="tree_dve", contig=True, split4=True,
    load_bufs=5, sq_bufs=2, inplace=False, bufs=3,
    store_batch=2, out_dt="f32",
)

_ODT_MAP = {"f32": F32, "f16": mybir.dt.float16, "bf16": mybir.dt.bfloat16}


def _build_bass(reps: int = 1, **overrides):
    """reps>1 repeats the whole per-core computation inside one NEFF —
    used by the timing harness to separate device time from launch/upload
    overhead ((T_reps - T_1) / (reps - 1))."""
    cfg = {**DEFAULT_CFG, **overrides}
    # Output precision: the selection pipeline stays fp32 (norm comparisons
    # must match the fp32 reference argmax), but the selected vectors may be
    # stored at reduced precision — the engines cast on write, halving
    # output HBM traffic. Host side casts back to fp32.
    odt = _ODT_MAP[cfg.get("out_dt", "f32")]
    cfg["_odt"] = odt
    nc = bass.Bass()
    x = nc.dram_tensor("x", [BL, H, W, C, A], F32, kind="ExternalInput")
    out = nc.dram_tensor("out", [BL, Ho, Wo, C, A], odt, kind="ExternalOutput")

    # [(b ho)=256, ph=2, pw=2, wo=32, ca=256]; (b, ho) merges because the
    # b stride (H*W*C*A) equals 32 * the ho stride (PH*W*C*A).
    xv = x.rearrange(
        "b (ho ph) (wo pw) c a -> (b ho) ph pw wo (c a)", ph=PH, pw=PW
    )
    # contiguous-load view: [(b ho), ph, w, ca] with w the full-res column.
    cfg["xvc"] = x.rearrange(
        "b (ho ph) w c a -> (b ho) ph w (c a)", ph=PH
    )
    ov = out.rearrange("b ho wo c a -> (b ho) wo (c a)")  # [256, 32, 256]

    with tile.TileContext(nc) as tc:
        with (
            tc.tile_pool(name="big", bufs=cfg["bufs"]) as big,
            tc.tile_pool(name="small", bufs=cfg["bufs"]) as small,
        ):
            sb = cfg.get("store_batch", 1)
            assert NWCH % sb == 0
            for _rep in range(reps):
                for rt in range(NRT):
                    r0 = rt * 128
                    for wq in range(NWCH // sb):
                        if sb == 1:
                            _group(nc, big, small, xv, ov, r0, wq * WCH, cfg)
                            continue
                        # Batch sb groups' outputs into one slab; a single
                        # store then has sb*WCH*CA*4 B contiguous per row.
                        oslab = big.tile(
                            [128, sb, WCH, CA], cfg["_odt"], name="oslab",
                            bufs=cfg.get("slab_bufs", 2),
                        )
                        for sub in range(sb):
                            _group(
                                nc, big, small, xv, ov, r0,
                                (wq * sb + sub) * WCH, cfg,
                                dst_out=oslab[:, sub],
                            )
                        oe = nc.scalar if cfg.get("out_act_ring") else nc.sync
                        oe.dma_start(
                            out=ov[
                                r0 : r0 + 128,
                                wq * sb * WCH : (wq + 1) * sb * WCH, :,
                            ],
                            in_=oslab,
                        )
    _split_multi_waits(nc)
    return nc


_NC_CACHE = None


def kernel(x: np.ndarray) -> np.ndarray:
    global _NC_CACHE
    assert x.shape == (B, H, W, C, A) and x.dtype == np.float32
    if _NC_CACHE is None:
        _NC_CACHE = _build_bass()
    nc = _NC_CACHE

    shards = [
        np.ascontiguousarray(x[i * BL : (i + 1) * BL]) for i in range(NCORES)
    ]
    in_maps = [{"x": s} for s in shards]
    res = run_bass_kernel_spmd(nc, in_maps, list(range(NCORES)))
    return np.concatenate(
        [np.asarray(r["out"]).astype(np.float32) for r in res.results], axis=0
    )



# revision 13
# speedup vs baseline: 1.3017x; 1.3017x over previous
"""CapsMaxPool Trainium2 kernel.

x: [B=64, H=64, W=64, C=32, A=8] fp32 capsules. For each 2x2 spatial window
and capsule c, pick the candidate position whose capsule vector has the
largest L2 norm (first-max-wins over the window in row-major (ph, pw) order)
and emit that 8-atom vector. Output: [B, 32, 32, 32, 8].

Strategy (per core; batch sharded 8 ways -> 8 examples/core):
  - Layout: spatial rows on SBUF partitions, (capsule, atom) on the free dim.
    One partition row = one (b, ho) output row.
  - Loads: one DMA per pooling row (ph) covering the full 2*wch window span,
    8 KiB contiguous per partition row on the DRAM side; the four window
    candidates are strided views of the loaded tile.
  - ScalarE squares everything (fp32; fp16/bf16 squares were measured to
    flip 346/2691 argmax decisions vs the fp32 reference, while the fp32
    pipeline is bit-exact and stable to +-6 ulp of any summation order).
  - Atom sums via a pairwise add tree on VectorE (tensor-tensor adds read
    2 elems/cycle via two SBUF ports vs tensor_reduce's 1).
  - VectorE builds strictly-greater masks against the running max
    (first-max-wins, identical to jnp.argmax), then copy_predicated
    overwrites a baseline copy of candidate 0, broadcasting each (wo, c)
    mask over the 8 atoms via a step-0 AP dim.
  - Output is stored as fp16 (engines cast on write; selection itself stays
    fp32-exact) and cast back to fp32 on host: halves the store-side HBM
    traffic, worth ~8 us/rep against the ~2e-2 rel-err budget (fp16
    quantization contributes rel L2 ~2e-4).
"""

import numpy as np

import concourse.bass as bass
import concourse.tile as tile
from concourse import mybir
from concourse.bass_utils import run_bass_kernel_spmd

B, H, W, C, A = 64, 64, 64, 32, 8
PH = PW = 2
NCORES = 8
BL = B // NCORES          # batches per core
Ho, Wo = H // PH, W // PW
CA = C * A                # 256
R = BL * Ho               # 256 partition rows per core ((b, ho) flattened)
WCH = 4                   # wo per tile group
NWCH = Wo // WCH
NRT = R // 128            # row tiles

F32 = mybir.dt.float32


def _split_multi_waits(nc):
    """Walrus on this toolchain encodes at most ONE sync wait per
    instruction; Tile attaches several. Hoist all-but-one wait into
    standalone InstEventSemaphore ops just before the instruction (same
    engine stream position => identical semantics)."""
    for fn in nc.m.functions:
        for bb in fn.blocks:
            new = []
            for ins in bb.instructions:
                si = ins.sync_info
                if si is not None and si.on_wait and len(si.on_wait) > 1:
                    waits = list(si.on_wait)
                    for j, w in enumerate(waits[:-1]):
                        new.append(mybir.InstEventSemaphore(
                            name=f"{ins.name}-hw{j}",
                            engine=ins.engine,
                            ins=[], outs=[],
                            sync_info=mybir.SyncInfo(on_wait=[w], on_update=[]),
                        ))
                    ins.sync_info = mybir.SyncInfo(
                        on_wait=[waits[-1]], on_update=list(si.on_update)
                    )
                new.append(ins)
            bb.instructions = new


def _bcast_atoms(ap):
    """View an AP with an extra step-0 trailing dim of size A."""
    return bass.AP(tensor=ap.tensor, offset=ap.offset, ap=list(ap.ap) + [[0, A]])


def _out_engine(nc, cfg):
    if cfg.get("out_gps_ring"):
        return nc.gpsimd
    return nc.scalar if cfg.get("out_act_ring") else nc.sync


def _group(nc, big, small, xv, ov, r0, w0, cfg, dst_out=None):
    WCH = cfg.get("wch", 4)
    NP = PH * PW
    if cfg.get("contig"):
        # One DMA per group: each partition row loads its full window span
        # (both ph rows x 2*WCH w positions) as two contiguous 8 KiB DRAM
        # chunks. Candidates are strided views of the loaded tile.
        xvc = cfg["xvc"]
        xq = big.tile(
            [128, PH, 2 * WCH, CA], F32, name="xq",
            bufs=cfg.get("load_bufs"),
        )
        if cfg.get("split4"):
            # 4 DMAs: one per (ph, w-half) — 4 KiB contiguous chunks with
            # candidate-level dependency granularity. Optionally spread
            # across both HWDGE rings (SP + Activation).
            for ph in range(PH):
                for wh in range(2):
                    eng = (
                        nc.scalar
                        if cfg.get("in_split_rings") and wh == 1
                        else nc.sync
                    )
                    eng.dma_start(
                        out=xq[:, ph, wh * WCH : (wh + 1) * WCH, :],
                        in_=xvc[
                            r0 : r0 + 128, ph,
                            2 * w0 + wh * WCH : 2 * w0 + (wh + 1) * WCH, :,
                        ],
                    )
        elif cfg.get("split_ph"):
            for ph in range(PH):
                nc.sync.dma_start(
                    out=xq[:, ph],
                    in_=xvc[r0 : r0 + 128, ph, 2 * w0 : 2 * (w0 + WCH), :],
                )
        else:
            nc.sync.dma_start(
                out=xq,
                in_=xvc[r0 : r0 + 128, :, 2 * w0 : 2 * (w0 + WCH), :],
            )
        xqv = xq[:].rearrange("p ph (wo pw) ca -> p ph pw wo ca", pw=PW)
        cand = lambda p: xqv[:, p // PW, p % PW]
    elif cfg.get("pair_load"):
        # xc laid out [part, w, cand, ca] so one DMA per ph row carries both
        # pw candidates: the DRAM side is then fully contiguous 8 KiB per
        # partition row, and the SBUF side stays a 3-dim AP.
        xcw = big.tile([128, WCH, NP, CA], F32, name="xcw")
        for ph in range(PH):
            nc.sync.dma_start(
                out=xcw[:, :, 2 * ph : 2 * ph + 2, :],
                in_=xv[r0 : r0 + 128, ph, :, w0 : w0 + WCH, :].rearrange(
                    "p pw w ca -> p w pw ca"
                ),
            )
        cand = lambda p: xcw[:, :, p, :]
    else:
        xc = big.tile(
            [128, NP, WCH, CA], F32, name="xc", bufs=cfg.get("load_bufs")
        )
        for p in range(NP):
            ph, pw = divmod(p, PW)
            nc.sync.dma_start(
                out=xc[:, p],
                in_=xv[r0 : r0 + 128, ph, pw, w0 : w0 + WCH, :],
            )
        cand = lambda p: xc[:, p]

    if cfg.get("loads_only"):
        return

    ODT = cfg.get("_odt", F32)

    if cfg.get("dma_only"):
        out_t = big.tile([128, WCH, CA], ODT, name="out_t")
        nc.scalar.copy(out_t, cand(0))
        _out_engine(nc, cfg).dma_start(
            out=ov[r0 : r0 + 128, w0 : w0 + WCH, :], in_=out_t
        )
        return

    sq = big.tile(
        [128, NP, WCH, CA], F32, name="sq", bufs=cfg.get("sq_bufs")
    )
    for p in range(NP):
        nc.scalar.activation(
            sq[:, p], cand(p), mybir.ActivationFunctionType.Square
        )

    s = small.tile([128, NP, WCH, C], F32, name="s")
    norm = cfg.get("norm", "tree_dve")
    if norm in ("tree_dve", "tree_gps"):
        l1_eng = nc.gpsimd if norm == "tree_gps" else nc.vector
        sqv = sq[:].rearrange(
            "p q w (c a2 two) -> p q w c a2 two", a2=A // 2, two=2
        )
        t4 = small.tile([128, NP, WCH, C, A // 2], F32, name="t4")
        l1_eng.tensor_add(t4, sqv[:, :, :, :, :, 0], sqv[:, :, :, :, :, 1])
        t4v = t4[:].rearrange("p q w c (b2 two) -> p q w c b2 two", two=2)
        t2 = small.tile([128, NP, WCH, C, A // 4], F32, name="t2")
        nc.vector.tensor_add(t2, t4v[:, :, :, :, :, 0], t4v[:, :, :, :, :, 1])
        t2v = t2[:].rearrange("p q w c (b1 two) -> p q w c b1 two", two=2)
        nc.vector.tensor_add(s, t2v[:, :, :, :, 0, 0], t2v[:, :, :, :, 0, 1])
    else:  # plain reduce on DVE
        nc.vector.tensor_reduce(
            s,
            sq[:].rearrange("p q w (c a) -> p q w c a", a=A),
            axis=mybir.AxisListType.X,
            op=mybir.AluOpType.add,
        )

    # Strict-greater masks vs the running max -> first-max-wins.
    m = [
        small.tile([128, WCH, C], mybir.dt.uint8, name=f"mask{i}", tag=f"mask{i}")
        for i in range(3)
    ]
    r01 = small.tile([128, WCH, C], F32, name="r01")
    r012 = small.tile([128, WCH, C], F32, name="r012")
    nc.vector.tensor_tensor(m[0], s[:, 1], s[:, 0], mybir.AluOpType.is_gt)
    nc.vector.tensor_max(r01, s[:, 0], s[:, 1])
    nc.vector.tensor_tensor(m[1], s[:, 2], r01, mybir.AluOpType.is_gt)
    nc.vector.tensor_max(r012, r01, s[:, 2])
    nc.vector.tensor_tensor(m[2], s[:, 3], r012, mybir.AluOpType.is_gt)

    # Select: overwrite the baseline wherever a later candidate strictly
    # beats the running max (mask broadcast over the 8 atoms via step-0).
    if dst_out is not None:
        dst_slice = dst_out
        nc.scalar.copy(dst_slice, cand(0))
    elif cfg.get("inplace"):
        dst_slice = cand(0)
    else:
        out_t = big.tile([128, WCH, CA], ODT, name="out_t")
        nc.scalar.copy(out_t, cand(0))
        dst_slice = out_t[:]
    dst = dst_slice.rearrange("p w (c a) -> p w c a", a=A)
    for p in range(1, NP):
        nc.vector.copy_predicated(
            dst,
            _bcast_atoms(m[p - 1][:]),
            cand(p).rearrange("p w (c a) -> p w c a", a=A),
        )

    if dst_out is None:
        _out_engine(nc, cfg).dma_start(
            out=ov[r0 : r0 + 128, w0 : w0 + WCH, :], in_=dst_slice
        )


DEFAULT_CFG = dict(
    norm="tree_dve", contig=True, split4=False, split_ph=True,
    load_bufs=6, sq_bufs=2, inplace=False, bufs=3,
    store_batch=4, out_dt="f16",
)

_ODT_MAP = {"f32": F32, "f16": mybir.dt.float16, "bf16": mybir.dt.bfloat16}


def _build_bass(reps: int = 1, **overrides):
    """reps>1 repeats the whole per-core computation inside one NEFF —
    used by the timing harness to separate device time from launch/upload
    overhead ((T_reps - T_1) / (reps - 1))."""
    cfg = {**DEFAULT_CFG, **overrides}
    # Output precision: the selection pipeline stays fp32 (norm comparisons
    # must match the fp32 reference argmax), but the selected vectors may be
    # stored at reduced precision — the engines cast on write, halving
    # output HBM traffic. Host side casts back to fp32.
    odt = _ODT_MAP[cfg.get("out_dt", "f32")]
    cfg["_odt"] = odt
    nc = bass.Bass()
    x = nc.dram_tensor("x", [BL, H, W, C, A], F32, kind="ExternalInput")
    out = nc.dram_tensor("out", [BL, Ho, Wo, C, A], odt, kind="ExternalOutput")

    # [(b ho)=256, ph=2, pw=2, wo=32, ca=256]; (b, ho) merges because the
    # b stride (H*W*C*A) equals 32 * the ho stride (PH*W*C*A).
    xv = x.rearrange(
        "b (ho ph) (wo pw) c a -> (b ho) ph pw wo (c a)", ph=PH, pw=PW
    )
    # contiguous-load view: [(b ho), ph, w, ca] with w the full-res column.
    cfg["xvc"] = x.rearrange(
        "b (ho ph) w c a -> (b ho) ph w (c a)", ph=PH
    )
    ov = out.rearrange("b ho wo c a -> (b ho) wo (c a)")  # [256, 32, 256]

    wch = cfg.get("wch", 4)
    nwch = Wo // wch
    with tile.TileContext(nc) as tc:
        with (
            tc.tile_pool(name="big", bufs=cfg["bufs"]) as big,
            tc.tile_pool(name="small", bufs=cfg["bufs"]) as small,
        ):
            sb = cfg.get("store_batch", 1)
            assert nwch % sb == 0
            for _rep in range(reps):
                for rt in range(NRT):
                    r0 = rt * 128
                    for wq in range(nwch // sb):
                        if sb == 1:
                            _group(nc, big, small, xv, ov, r0, wq * wch, cfg)
                            continue
                        # Batch sb groups' outputs into one slab; a single
                        # store then has sb*wch*CA*dt B contiguous per row.
                        oslab = big.tile(
                            [128, sb, wch, CA], cfg["_odt"], name="oslab",
                            bufs=cfg.get("slab_bufs", 2),
                        )
                        for sub in range(sb):
                            _group(
                                nc, big, small, xv, ov, r0,
                                (wq * sb + sub) * wch, cfg,
                                dst_out=oslab[:, sub],
                            )
                        _out_engine(nc, cfg).dma_start(
                            out=ov[
                                r0 : r0 + 128,
                                wq * sb * wch : (wq + 1) * sb * wch, :,
                            ],
                            in_=oslab,
                        )
    _split_multi_waits(nc)
    return nc


_NC_CACHE = None


def kernel(x: np.ndarray) -> np.ndarray:
    global _NC_CACHE
    assert x.shape == (B, H, W, C, A) and x.dtype == np.float32
    if _NC_CACHE is None:
        _NC_CACHE = _build_bass()
    nc = _NC_CACHE

    shards = [
        np.ascontiguousarray(x[i * BL : (i + 1) * BL]) for i in range(NCORES)
    ]
    in_maps = [{"x": s} for s in shards]
    res = run_bass_kernel_spmd(nc, in_maps, list(range(NCORES)))
    return np.concatenate(
        [np.asarray(r["out"]).astype(np.float32) for r in res.results], axis=0
    )

